# revision 18
# baseline (speedup 1.0000x reference)
"""Trainium2 Bass kernel for causal self-attention with RoPE.

Sharding: 8 cores = batch(2) x head-group(4).  Each core computes qkv + RoPE +
flash attention for its 4 heads of its batch, then an AllToAll inside each
4-core batch group re-shards attention output to token blocks (512 rows) for
the output projection.  All outputs are disjoint across cores; the host only
concatenates / transposes / casts.
"""

import sys
import types

import ml_dtypes
import numpy as np

# ---------------------------------------------------------------------------
# NTFF profiling shim: the agent image's antenv lacks axon_hooks; register the
# hook module + ctypes-driven profiler so run_bass_kernel_spmd(trace=True)
# works.  Harmless if profiling is never requested.
try:
    import antenv

    if "antenv.axon_hooks" not in sys.modules:
        _shim = types.ModuleType("antenv.axon_hooks")
        _shim._hook = None

        def _set_hook(h):
            _shim._hook = h

        def _get_hook():
            return _shim._hook

        _shim.set_axon_ntff_profile_hook = _set_hook
        _shim.get_axon_ntff_profile_hook = _get_hook
        sys.modules["antenv.axon_hooks"] = _shim
        antenv.axon_hooks = _shim
        try:
            from trn_agent_boot.trn_boot import _ntff_profile_via_ctypes

            _set_hook(_ntff_profile_via_ctypes("/opt/axon/libaxon_pjrt.so"))
        except Exception:
            pass
except Exception:
    pass

import concourse.bass as bass
import concourse.mybir as mybir
import concourse.tile as tile
from concourse import bacc
from concourse.bass_utils import run_bass_kernel_spmd
from concourse.masks import make_identity

B, T, C, H, D = 2, 2048, 1024, 16, 64
P = 128
KO = C // P            # 8 contraction blocks
HL = 4                 # heads per core
TBLK = T // 4          # 512 token rows per core after re-shard
QCW = 1024             # attention q-chunk width
NEG = -1.0e30
BF16 = ml_dtypes.bfloat16

_NC_CACHE = None
_LAST = None


def _segs(lo, hi):
    """Split [lo, hi) at 512 boundaries (PSUM bank-aligned matmul chunks)."""
    out = []
    c = lo
    while c < hi:
        nxt = min(hi, (c // 512 + 1) * 512)
        out.append((c, nxt - c))
        c = nxt
    return out


def _build():
    dt = mybir.dt
    f32, bf16 = dt.float32, dt.bfloat16
    nc = bacc.Bacc(None, target_bir_lowering=False)

    xT = nc.dram_tensor("xT", [C, T], bf16, kind="ExternalInput")
    wl = nc.dram_tensor("wl", [C, 768], bf16, kind="ExternalInput")
    wp = nc.dram_tensor("wp", [2 * C, C], bf16, kind="ExternalInput")
    ct = nc.dram_tensor("ct", [P, T], bf16, kind="ExternalInput")
    st = nc.dram_tensor("st", [P, T], bf16, kind="ExternalInput")
    mk = nc.dram_tensor("mk", [P, P], bf16, kind="ExternalInput")
    pm = nc.dram_tensor("pm", [P, P], bf16, kind="ExternalInput")
    y_t = nc.dram_tensor("y_t", [C, TBLK], f32, kind="ExternalOutput")
    pk = nc.dram_tensor("pk", [2 * P, T], bf16, kind="ExternalOutput")
    pv = nc.dram_tensor("pv", [T, 2 * P], f32, kind="ExternalOutput")

    with tile.TileContext(nc) as tc:
        with (
            tc.tile_pool(name="const", bufs=1) as cpool,
            tc.tile_pool(name="big", bufs=1) as bpool,
            tc.tile_pool(name="work", bufs=3) as wpool,
            tc.tile_pool(name="norm", bufs=2) as npool,
            tc.tile_pool(name="dram", bufs=1, space="DRAM") as dpool,
        ):
            # ---- load inputs / constants ----
            xT_sb = bpool.tile([P, KO, T], bf16)
            nc.sync.dma_start(xT_sb[:], xT.rearrange("(ko p) t -> p ko t", p=P))
            wl_sb = bpool.tile([P, KO, 768], bf16)
            nc.sync.dma_start(wl_sb[:], wl.rearrange("(ko p) n -> p ko n", p=P))
            wp_sb = bpool.tile([P, 2 * KO, C], bf16)
            nc.sync.dma_start(wp_sb[:], wp.rearrange("(ko p) n -> p ko n", p=P))
            ct_sb = cpool.tile([P, T], bf16)
            nc.sync.dma_start(ct_sb[:], ct[:])
            st_sb = cpool.tile([P, T], bf16)
            nc.sync.dma_start(st_sb[:], st[:])
            mk_sb = cpool.tile([P, P], bf16)
            nc.sync.dma_start(mk_sb[:], mk[:])
            ident = cpool.tile([P, P], bf16)
            make_identity(nc, ident[:])
            pm_sb = cpool.tile([P, P], bf16)
            nc.sync.dma_start(pm_sb[:], pm[:])

            # roped q^T,k^T: col-blocks 0,1 = q (head pairs), 2,3 = k
            qkT_sb = bpool.tile([P, 4, T], bf16)
            # v with ones column per head: [128 t, 16 tb, 4*65]
            v_sb = bpool.tile([P, 16, HL * 65], bf16)
            nc.gpsimd.memset(v_sb[:], 1.0)
            # normalized attention output^T (2 head blocks x T)
            aT_sb = bpool.tile([P, 2, T], bf16)

            # ---- phase 1: qk^T matmul + rope;  v matmul ----
            with (
                tc.tile_pool(name="psqk", bufs=1, space="PSUM") as psqk,
                tc.tile_pool(name="psperm", bufs=2, space="PSUM") as psperm,
                tc.tile_pool(name="psv", bufs=2, space="PSUM") as psv,
            ):
                for cb in range(4):
                    qk_ps = psqk.tile([P, T], f32, name=f"qkps{cb}", tag="qk")
                    for ko in range(KO):
                        for t0 in range(0, T, 512):
                            nc.tensor.matmul(
                                qk_ps[:, t0 : t0 + 512],
                                lhsT=wl_sb[:, ko, cb * P : (cb + 1) * P],
                                rhs=xT_sb[:, ko, t0 : t0 + 512],
                                start=(ko == 0),
                                stop=(ko == KO - 1),
                            )
                    cprod = wpool.tile([P, T], bf16, name=f"cp{cb}", tag="cprod")
                    zt = wpool.tile([P, T], bf16, name=f"zt{cb}", tag="zt")
                    nc.vector.tensor_mul(cprod[:], qk_ps[:], ct_sb[:])
                    nc.vector.tensor_mul(zt[:], qk_ps[:], st_sb[:])
                    for c0 in range(0, T, 512):
                        pp = psperm.tile(
                            [P, 512], f32, name=f"pp{cb}_{c0}", tag="perm"
                        )
                        nc.tensor.matmul(
                            pp[:],
                            lhsT=pm_sb[:],
                            rhs=zt[:, c0 : c0 + 512],
                            start=True,
                            stop=True,
                        )
                        nc.vector.tensor_add(
                            qkT_sb[:, cb, c0 : c0 + 512],
                            pp[:],
                            cprod[:, c0 : c0 + 512],
                        )
                    if cb >= 2:
                        nc.sync.dma_start(
                            pk[(cb - 2) * P : (cb - 1) * P, :], qkT_sb[:, cb, :]
                        )

                for rb in range(16):
                    v_ps = psv.tile([P, 256], f32, name=f"vps{rb}", tag="v")
                    for ko in range(KO):
                        nc.tensor.matmul(
                            v_ps[:],
                            lhsT=xT_sb[:, ko, rb * P : (rb + 1) * P],
                            rhs=wl_sb[:, ko, 512:768],
                            start=(ko == 0),
                            stop=(ko == KO - 1),
                        )
                    vf = wpool.tile([P, 256], f32, name=f"vf{rb}", tag="vf")
                    nc.scalar.copy(vf[:], v_ps[:])
                    nc.sync.dma_start(pv[rb * P : (rb + 1) * P, :], vf[:])
                    nc.vector.tensor_copy(
                        v_sb[:, rb, :]
                        .rearrange("p (h e) -> p h e", e=65)[:, :, 0:64],
                        v_ps[:].rearrange("p (h d) -> p h d", d=64),
                    )

            # ---- phase 2: flash attention per (head, q-chunk) ----
            with (
                tc.tile_pool(name="pss", bufs=2, space="PSUM") as pss,
                tc.tile_pool(name="psao", bufs=2, space="PSUM") as psao,
            ):
                for h in range(HL):
                    hb, hi = h // 2, (h % 2) * 64
                    for qc in range(T // QCW):
                        ao_ps = psao.tile(
                            [65, QCW], f32, name=f"ao{h}_{qc}", tag="ao"
                        )
                        kmax = (qc + 1) * QCW // P
                        for kb in range(kmax):
                            q_lo = max(qc * QCW, kb * P)
                            off = q_lo - qc * QCW
                            s_ps = pss.tile(
                                [P, QCW], f32, name=f"s{h}_{qc}_{kb}", tag="s"
                            )
                            diag = kb * P >= qc * QCW
                            for j, (c0, cw) in enumerate(_segs(off, QCW)):
                                nc.tensor.matmul(
                                    s_ps[:, c0 : c0 + cw],
                                    lhsT=qkT_sb[
                                        hi : hi + 64, 2 + hb, kb * P : (kb + 1) * P
                                    ],
                                    rhs=qkT_sb[
                                        hi : hi + 64,
                                        hb,
                                        qc * QCW + c0 : qc * QCW + c0 + cw,
                                    ],
                                    start=True,
                                    stop=not (diag and j == 0),
                                )
                            if diag:
                                nc.tensor.matmul(
                                    s_ps[:, off : off + P],
                                    lhsT=ident[:],
                                    rhs=mk_sb[:],
                                    start=False,
                                    stop=True,
                                )
                            pt = wpool.tile([P, QCW], bf16, name=f"pt{kb}", tag="pt")
                            nc.scalar.activation(
                                pt[:, off:],
                                s_ps[:, off:],
                                mybir.ActivationFunctionType.Exp,
                                scale=0.125,
                            )
                            for c0, cw in _segs(off, QCW):
                                nc.tensor.matmul(
                                    ao_ps[:, c0 : c0 + cw],
                                    lhsT=v_sb[:, kb, h * 65 : (h + 1) * 65],
                                    rhs=pt[:, c0 : c0 + cw],
                                    start=(kb == 0),
                                    stop=(kb == kmax - 1),
                                )
                        rs = npool.tile([1, QCW], f32, name=f"rs{h}{qc}", tag="rs")
                        nc.vector.reciprocal(rs[:], ao_ps[64:65, :])
                        bc = npool.tile([64, QCW], f32, name=f"bc{h}{qc}", tag="bc")
                        nc.gpsimd.partition_broadcast(bc[:], rs[:])
                        nc.vector.tensor_mul(
                            aT_sb[hi : hi + 64, hb, qc * QCW : (qc + 1) * QCW],
                            ao_ps[0:64, :],
                            bc[:],
                        )

            # ---- phase 3: all-to-all re-shard + projection ----
            a2a_in = dpool.tile([8 * 256, TBLK], bf16)
            a2a_out = dpool.tile([8 * 256, TBLK], bf16)
            for s in range(8):
                j = s % 4
                for blk in range(2):
                    nc.sync.dma_start(
                        a2a_in[s * 256 + blk * P : s * 256 + (blk + 1) * P, :],
                        aT_sb[:, blk, j * TBLK : (j + 1) * TBLK],
                    )
            nc.gpsimd.collective_compute(
                "AllToAll",
                mybir.AluOpType.bypass,
                replica_groups=[[0, 1, 2, 3, 4, 5, 6, 7]],
                ins=[a2a_in.opt()],
                outs=[a2a_out.opt()],
            )
            with tc.tile_pool(name="psy", bufs=1, space="PSUM") as psy:
                y_ps = [
                    psy.tile([P, TBLK], f32, name=f"y{cb2}", tag=f"y{cb2}")
                    for cb2 in range(8)
                ]
                for ko2 in range(16):
                    rt = wpool.tile([P, TBLK], bf16, name=f"rt{ko2}", tag="rt")
                    nc.sync.dma_start(rt[:], a2a_out[ko2 * P : (ko2 + 1) * P, :])
                    for cb2 in range(8):
                        nc.tensor.matmul(
                            y_ps[cb2][:],
                            lhsT=wp_sb[:, ko2, cb2 * P : (cb2 + 1) * P],
                            rhs=rt[:],
                            start=(ko2 == 0),
                            stop=(ko2 == 15),
                        )
                for cb2 in range(8):
                    yf = wpool.tile([P, TBLK], f32, name=f"yf{cb2}", tag="yf")
                    nc.scalar.copy(yf[:], y_ps[cb2][:])
                    nc.sync.dma_start(y_t[cb2 * P : (cb2 + 1) * P, :], yf[:])

    nc.compile()
    return nc


def _get_nc():
    global _NC_CACHE
    if _NC_CACHE is None:
        _NC_CACHE = _build()
    return _NC_CACHE


def _rope_perm():
    """Column permutation (within one head, 64 cols) to de-interleaved
    [r(32) | i(32)] order."""
    return np.concatenate([np.arange(0, D, 2), np.arange(1, D, 2)])


def kernel(x, freqs_cos, freqs_sin, w_qkv, w_proj):
    x = np.asarray(x)
    freqs_cos = np.asarray(freqs_cos)
    freqs_sin = np.asarray(freqs_sin)
    w_qkv = np.asarray(w_qkv)
    w_proj = np.asarray(w_proj)

    nc = _get_nc()
    perm = _rope_perm()
    wq = w_qkv[:, 0:C]
    wk = w_qkv[:, C : 2 * C]
    wv = w_qkv[:, 2 * C : 3 * C]

    cosT = freqs_cos.T.astype(np.float32)  # (32, T)
    sinT = freqs_sin.T.astype(np.float32)
    ct_np = np.tile(cosT, (4, 1)).astype(BF16)  # every 32-row gets cos
    st_np = np.concatenate([sinT, -sinT, sinT, -sinT], axis=0).astype(BF16)

    kk, qq = np.meshgrid(np.arange(P), np.arange(P), indexing="ij")
    mk_np = np.where(qq >= kk, 0.0, NEG).astype(BF16)

    pm_np = np.zeros((P, P), dtype=BF16)
    pm_np[np.arange(P) ^ 32, np.arange(P)] = 1.0

    wp_np_b = []
    for b in range(B):
        wpb = np.zeros((2 * C, C), dtype=BF16)
        wpb[b * C : (b + 1) * C, :] = w_proj.astype(BF16)
        wp_np_b.append(wpb)

    in_maps = []
    for core in range(8):
        b, g = core // 4, core % 4
        heads = np.arange(4 * g, 4 * g + 4)
        # q/k columns: per head-pair block, [h0-perm | h1-perm]
        qcols = np.concatenate([h * D + perm for h in heads])
        vcols = np.concatenate([h * D + np.arange(D) for h in heads])
        wl_np = np.concatenate(
            [wq[:, qcols], wk[:, qcols], wv[:, vcols]], axis=1
        ).astype(BF16)
        xT_np = np.ascontiguousarray(x[b].T).astype(BF16)
        in_maps.append(
            {
                "xT": xT_np,
                "wl": wl_np,
                "wp": wp_np_b[b],
                "ct": ct_np,
                "st": st_np,
                "mk": mk_np,
                "pm": pm_np,
            }
        )

    res = run_bass_kernel_spmd(nc, in_maps, core_ids=list(range(8)))
    global _LAST
    _LAST = res

    y = np.empty((B, T, C), dtype=np.float32)
    present_k = np.empty((B, T, H, D), dtype=np.float32)
    present_v = np.empty((B, T, H, D), dtype=np.float32)
    inv = np.argsort(perm)
    for core in range(8):
        b, g = core // 4, core % 4
        r = res.results[core]
        y[b, g * TBLK : (g + 1) * TBLK, :] = r["y_t"].T
        pk = r["pk"].astype(np.float32)  # (256, T) rope-layout
        for u in range(4):
            h = 4 * g + u
            blk, off = u // 2, (u % 2) * 64
            kh = pk[blk * P + off : blk * P + off + 64, :]  # (64, T) [r|i]
            present_k[b, :, h, :] = kh.T[:, inv]
        pv = r["pv"]  # (T, 256) fp32
        present_v[b, :, 4 * g : 4 * g + 4, :] = pv.reshape(T, 4, D)
    return (y, present_k, present_v)


# revision 30
# speedup vs baseline: 1.2186x; 1.2186x over previous
"""Trainium2 Bass kernel for causal self-attention with RoPE.

Sharding: 8 cores = batch(2) x head-group(4).  Each core computes qkv + RoPE +
flash attention for its 4 heads of its batch, then an AllToAll inside each
4-core batch group re-shards attention output to token blocks (512 rows) for
the output projection.  All outputs are disjoint across cores; the host only
concatenates / transposes / casts.
"""

import sys
import types

import ml_dtypes
import numpy as np

# ---------------------------------------------------------------------------
# NTFF profiling shim: the agent image's antenv lacks axon_hooks; register the
# hook module + ctypes-driven profiler so run_bass_kernel_spmd(trace=True)
# works.  Harmless if profiling is never requested.
try:
    import antenv

    if "antenv.axon_hooks" not in sys.modules:
        _shim = types.ModuleType("antenv.axon_hooks")
        _shim._hook = None

        def _set_hook(h):
            _shim._hook = h

        def _get_hook():
            return _shim._hook

        _shim.set_axon_ntff_profile_hook = _set_hook
        _shim.get_axon_ntff_profile_hook = _get_hook
        sys.modules["antenv.axon_hooks"] = _shim
        antenv.axon_hooks = _shim
        try:
            from trn_agent_boot.trn_boot import _ntff_profile_via_ctypes

            _set_hook(_ntff_profile_via_ctypes("/opt/axon/libaxon_pjrt.so"))
        except Exception:
            pass
except Exception:
    pass

import concourse.bass as bass
import concourse.mybir as mybir
import concourse.tile as tile
from concourse import bacc
from concourse.bass_utils import run_bass_kernel_spmd
from concourse.masks import make_identity

B, T, C, H, D = 2, 2048, 1024, 16, 64
P = 128
KO = C // P            # 8 contraction blocks
HL = 4                 # heads per core
TBLK = T // 4          # 512 token rows per core after re-shard
QCW = 1024             # attention q-chunk width
NEG = -1.0e30
BF16 = ml_dtypes.bfloat16

_NC_CACHE = None
_LAST = None


def _segs(lo, hi):
    """Split [lo, hi) at 512 boundaries (PSUM bank-aligned matmul chunks)."""
    out = []
    c = lo
    while c < hi:
        nxt = min(hi, (c // 512 + 1) * 512)
        out.append((c, nxt - c))
        c = nxt
    return out


def _build():
    dt = mybir.dt
    f32, bf16 = dt.float32, dt.bfloat16
    nc = bacc.Bacc(None, target_bir_lowering=False)

    xT = nc.dram_tensor("xT", [C, T], bf16, kind="ExternalInput")
    wl = nc.dram_tensor("wl", [C, 768], bf16, kind="ExternalInput")
    wp = nc.dram_tensor("wp", [2 * C, C], bf16, kind="ExternalInput")
    ct = nc.dram_tensor("ct", [P, T], bf16, kind="ExternalInput")
    st = nc.dram_tensor("st", [P, T], bf16, kind="ExternalInput")
    mk = nc.dram_tensor("mk", [P, P], bf16, kind="ExternalInput")
    pm = nc.dram_tensor("pm", [P, P], bf16, kind="ExternalInput")
    y_t = nc.dram_tensor("y_t", [C, TBLK], f32, kind="ExternalOutput")
    pk = nc.dram_tensor("pk", [2 * P, T], bf16, kind="ExternalOutput")
    pv = nc.dram_tensor("pv", [T, 2 * P], f32, kind="ExternalOutput")

    with tile.TileContext(nc) as tc:
        with (
            tc.tile_pool(name="const", bufs=1) as cpool,
            tc.tile_pool(name="big", bufs=1) as bpool,
            tc.tile_pool(name="work", bufs=3) as wpool,
            tc.tile_pool(name="norm", bufs=2) as npool,
            tc.tile_pool(name="dram", bufs=1, space="DRAM") as dpool,
        ):
            # ---- load inputs / constants ----
            xT_r = xT.rearrange("(ko p) t -> p ko t", p=P)
            xT_sb = bpool.tile([P, KO, T], bf16)
            for ko in range(KO):
                nc.sync.dma_start(xT_sb[:, ko, :], xT_r[:, ko, :])
            wl_r = wl.rearrange("(ko p) n -> p ko n", p=P)
            wl_sb = bpool.tile([P, KO, 768], bf16)
            for ko in range(KO):
                nc.sync.dma_start(wl_sb[:, ko, :], wl_r[:, ko, :])
            ct_sb = cpool.tile([P, T], bf16)
            nc.sync.dma_start(ct_sb[:], ct[:])
            st_sb = cpool.tile([P, T], bf16)
            nc.sync.dma_start(st_sb[:], st[:])
            mk_sb = cpool.tile([P, P], bf16)
            nc.sync.dma_start(mk_sb[:], mk[:])
            ident = cpool.tile([P, P], bf16)
            make_identity(nc, ident[:])
            pm_sb = cpool.tile([P, P], bf16)
            nc.sync.dma_start(pm_sb[:], pm[:])

            # roped q^T,k^T: col-blocks 0,1 = q (head pairs), 2,3 = k
            qkT_sb = bpool.tile([P, 4, T], bf16)
            # v with ones column per head: [128 t, 16 tb, 4*65]
            v_sb = bpool.tile([P, 16, HL * 65], bf16)
            nc.gpsimd.memset(v_sb[:], 1.0)
            # normalized attention output^T (2 head blocks x T)
            aT_sb = bpool.tile([P, 2, T], bf16)

            # ---- phase 1: qk^T matmul + rope;  v matmul ----
            with (
                tc.tile_pool(name="psqk", bufs=1, space="PSUM") as psqk,
                tc.tile_pool(name="psperm", bufs=2, space="PSUM") as psperm,
                tc.tile_pool(name="psv", bufs=2, space="PSUM") as psv,
            ):
                for cb in range(4):
                    qk_ps = psqk.tile([P, T], f32, name=f"qkps{cb}", tag="qk")
                    for ko in range(KO):
                        for t0 in range(0, T, 512):
                            nc.tensor.matmul(
                                qk_ps[:, t0 : t0 + 512],
                                lhsT=wl_sb[:, ko, cb * P : (cb + 1) * P],
                                rhs=xT_sb[:, ko, t0 : t0 + 512],
                                start=(ko == 0),
                                stop=(ko == KO - 1),
                            )
                    cprod = wpool.tile([P, T], bf16, name=f"cp{cb}", tag="cprod")
                    zt = wpool.tile([P, T], bf16, name=f"zt{cb}", tag="zt")
                    nc.vector.tensor_mul(cprod[:], qk_ps[:], ct_sb[:])
                    nc.vector.tensor_mul(zt[:], qk_ps[:], st_sb[:])
                    for c0 in range(0, T, 512):
                        pp = psperm.tile(
                            [P, 512], f32, name=f"pp{cb}_{c0}", tag="perm"
                        )
                        nc.tensor.matmul(
                            pp[:],
                            lhsT=pm_sb[:],
                            rhs=zt[:, c0 : c0 + 512],
                            start=True,
                            stop=True,
                        )
                        nc.vector.tensor_add(
                            qkT_sb[:, cb, c0 : c0 + 512],
                            pp[:],
                            cprod[:, c0 : c0 + 512],
                        )
                    if cb >= 2:
                        nc.sync.dma_start(
                            pk[(cb - 2) * P : (cb - 1) * P, :], qkT_sb[:, cb, :]
                        )

                for rb in range(16):
                    v_ps = psv.tile([P, 256], f32, name=f"vps{rb}", tag="v")
                    for ko in range(KO):
                        nc.tensor.matmul(
                            v_ps[:],
                            lhsT=xT_sb[:, ko, rb * P : (rb + 1) * P],
                            rhs=wl_sb[:, ko, 512:768],
                            start=(ko == 0),
                            stop=(ko == KO - 1),
                        )
                    vf = wpool.tile([P, 256], f32, name=f"vf{rb}", tag="vf")
                    nc.scalar.copy(vf[:], v_ps[:])
                    nc.sync.dma_start(pv[rb * P : (rb + 1) * P, :], vf[:])
                    nc.vector.tensor_copy(
                        v_sb[:, rb, :]
                        .rearrange("p (h e) -> p h e", e=65)[:, :, 0:64],
                        v_ps[:].rearrange("p (h d) -> p h d", d=64),
                    )

            # ---- phase 2: flash attention, head-pair outer.  After pair hp
            # finishes, its 128 channel rows for every token block go out in
            # AllToAll call hp; call 0 overlaps pair-1 attention, call 1
            # overlaps the first proj stage. ----
            a2a_in = [dpool.tile([8 * P, TBLK], bf16, name=f"ai{i}") for i in range(2)]
            a2a_out = [dpool.tile([8 * P, TBLK], bf16, name=f"aq{i}") for i in range(2)]
            # unnormalized attention outputs (bf16): slot per (h, qc)
            aou_sb = bpool.tile([64, 2 * HL, QCW], bf16)
            with (
                tc.tile_pool(name="pss", bufs=3, space="PSUM") as pss,
                tc.tile_pool(name="psao", bufs=1, space="PSUM") as psao,
            ):
                for hp in range(2):
                    rs_hp = npool.tile([P, QCW], f32, name=f"rs{hp}", tag="rs")
                    nc.gpsimd.memset(rs_hp[:], 1.0)
                    for hi2 in range(2):
                        h = 2 * hp + hi2
                        hi = hi2 * 64
                        for qc in range(T // QCW):
                            ao_ps = psao.tile(
                                [65, QCW], f32, name=f"ao{h}_{qc}", tag="ao"
                            )
                            kmax = (qc + 1) * QCW // P
                            for kb in range(kmax):
                                q_lo = max(qc * QCW, kb * P)
                                off = q_lo - qc * QCW
                                s_ps = pss.tile(
                                    [P, QCW], f32, name=f"s{h}_{qc}_{kb}", tag="s"
                                )
                                diag = kb * P >= qc * QCW
                                for j, (c0, cw) in enumerate(_segs(off, QCW)):
                                    nc.tensor.matmul(
                                        s_ps[:, c0 : c0 + cw],
                                        lhsT=qkT_sb[
                                            hi : hi + 64, 2 + hp, kb * P : (kb + 1) * P
                                        ],
                                        rhs=qkT_sb[
                                            hi : hi + 64,
                                            hp,
                                            qc * QCW + c0 : qc * QCW + c0 + cw,
                                        ],
                                        start=True,
                                        stop=not (diag and j == 0),
                                    )
                                if diag:
                                    nc.tensor.matmul(
                                        s_ps[:, off : off + P],
                                        lhsT=ident[:],
                                        rhs=mk_sb[:],
                                        start=False,
                                        stop=True,
                                    )
                                pt = wpool.tile(
                                    [P, QCW], bf16, name=f"pt{kb}", tag="pt"
                                )
                                nc.scalar.activation(
                                    pt[:, off:],
                                    s_ps[:, off:],
                                    mybir.ActivationFunctionType.Exp,
                                    scale=0.125,
                                )
                                for c0, cw in _segs(off, QCW):
                                    nc.tensor.matmul(
                                        ao_ps[:, c0 : c0 + cw],
                                        lhsT=v_sb[:, kb, h * 65 : (h + 1) * 65],
                                        rhs=pt[:, c0 : c0 + cw],
                                        start=(kb == 0),
                                        stop=(kb == kmax - 1),
                                    )
                            slot = 2 * hi2 + qc
                            nc.vector.tensor_copy(
                                rs_hp[32 * slot : 32 * slot + 1, :], ao_ps[64:65, :]
                            )
                            nc.vector.tensor_copy(
                                aou_sb[:, 2 * h + qc, :], ao_ps[0:64, :]
                            )
                    rcp = npool.tile([P, QCW], bf16, name=f"rcp{hp}", tag="rcp")
                    with nc.allow_low_precision(reason="softmax denom, 2e-2 gate"):
                        nc.vector.reciprocal(rcp[:], rs_hp[:])
                    # partition_broadcast always reads the tile's physical
                    # partition 0 -> stage each row into its own tile first
                    for hi2 in range(2):
                        h = 2 * hp + hi2
                        for qc in range(T // QCW):
                            slot = 2 * hi2 + qc
                            bcs = npool.tile(
                                [1, QCW], bf16, name=f"bcs{h}{qc}", tag="bcs"
                            )
                            nc.vector.tensor_copy(
                                bcs[:], rcp[32 * slot : 32 * slot + 1, :]
                            )
                            bc = npool.tile(
                                [64, QCW], bf16, name=f"bc{h}{qc}", tag="bc"
                            )
                            nc.gpsimd.partition_broadcast(bc[:], bcs[:])
                            nc.vector.tensor_mul(
                                aT_sb[
                                    64 * hi2 : 64 * hi2 + 64,
                                    hp,
                                    qc * QCW : (qc + 1) * QCW,
                                ],
                                aou_sb[:, 2 * h + qc, :],
                                bc[:],
                            )
                    for s in range(8):
                        nc.sync.dma_start(
                            a2a_in[hp][s * P : (s + 1) * P, :],
                            aT_sb[:, hp, (s % 4) * TBLK : (s % 4 + 1) * TBLK],
                        )
                    nc.gpsimd.collective_compute(
                        "AllToAll",
                        mybir.AluOpType.bypass,
                        replica_groups=[[0, 1, 2, 3, 4, 5, 6, 7]],
                        ins=[a2a_in[hp].opt()],
                        outs=[a2a_out[hp].opt()],
                    )

            # ---- phase 3: projection; stage hp consumes a2a call hp ----
            with tc.tile_pool(name="psy", bufs=1, space="PSUM") as psy:
                y_ps = [
                    psy.tile([P, TBLK], f32, name=f"y{cb2}", tag=f"y{cb2}")
                    for cb2 in range(8)
                ]
                for hp in range(2):
                    for s8 in range(8):
                        ko2 = hp * 8 + s8
                        rt = wpool.tile([P, TBLK], bf16, name=f"rt{ko2}", tag="rt")
                        nc.sync.dma_start(
                            rt[:], a2a_out[hp][s8 * P : (s8 + 1) * P, :]
                        )
                        wt = wpool.tile([P, C], bf16, name=f"wt{ko2}", tag="wt")
                        nc.sync.dma_start(wt[:], wp[ko2 * P : (ko2 + 1) * P, :])
                        for cb2 in range(8):
                            nc.tensor.matmul(
                                y_ps[cb2][:],
                                lhsT=wt[:, cb2 * P : (cb2 + 1) * P],
                                rhs=rt[:],
                                start=(ko2 == 0),
                                stop=(ko2 == 15),
                            )
                for cb2 in range(8):
                    yf = wpool.tile([P, TBLK], f32, name=f"yf{cb2}", tag="yf")
                    nc.scalar.copy(yf[:], y_ps[cb2][:])
                    nc.sync.dma_start(y_t[cb2 * P : (cb2 + 1) * P, :], yf[:])

    nc.compile()
    return nc


def _get_nc():
    global _NC_CACHE
    if _NC_CACHE is None:
        _NC_CACHE = _build()
    return _NC_CACHE


def _rope_perm():
    """Column permutation (within one head, 64 cols) to de-interleaved
    [r(32) | i(32)] order."""
    return np.concatenate([np.arange(0, D, 2), np.arange(1, D, 2)])


def kernel(x, freqs_cos, freqs_sin, w_qkv, w_proj):
    x = np.asarray(x)
    freqs_cos = np.asarray(freqs_cos)
    freqs_sin = np.asarray(freqs_sin)
    w_qkv = np.asarray(w_qkv)
    w_proj = np.asarray(w_proj)

    nc = _get_nc()
    perm = _rope_perm()
    wq = w_qkv[:, 0:C]
    wk = w_qkv[:, C : 2 * C]
    wv = w_qkv[:, 2 * C : 3 * C]

    cosT = freqs_cos.T.astype(np.float32)  # (32, T)
    sinT = freqs_sin.T.astype(np.float32)
    ct_np = np.tile(cosT, (4, 1)).astype(BF16)  # every 32-row gets cos
    st_np = np.concatenate([sinT, -sinT, sinT, -sinT], axis=0).astype(BF16)

    kk, qq = np.meshgrid(np.arange(P), np.arange(P), indexing="ij")
    mk_np = np.where(qq >= kk, 0.0, NEG).astype(BF16)

    pm_np = np.zeros((P, P), dtype=BF16)
    pm_np[np.arange(P) ^ 32, np.arange(P)] = 1.0

    # wp rows follow the a2a receive layout: row hp*1024 + s*128 + r holds
    # w_proj[(s%4)*256 + hp*128 + r] for same-batch senders s, zero otherwise.
    wpf = w_proj.astype(BF16)
    wp_np_b = []
    for b in range(B):
        wpb = np.zeros((2 * C, C), dtype=BF16)
        for hp in range(2):
            for s in range(8):
                if s // 4 != b:
                    continue
                dst = hp * C + s * P
                src = (s % 4) * 256 + hp * P
                wpb[dst : dst + P, :] = wpf[src : src + P, :]
        wp_np_b.append(wpb)

    in_maps = []
    for core in range(8):
        b, g = core // 4, core % 4
        heads = np.arange(4 * g, 4 * g + 4)
        # q/k columns: per head-pair block, [h0-perm | h1-perm]
        qcols = np.concatenate([h * D + perm for h in heads])
        vcols = np.concatenate([h * D + np.arange(D) for h in heads])
        wl_np = np.concatenate(
            [wq[:, qcols], wk[:, qcols], wv[:, vcols]], axis=1
        ).astype(BF16)
        xT_np = np.ascontiguousarray(x[b].T).astype(BF16)
        in_maps.append(
            {
                "xT": xT_np,
                "wl": wl_np,
                "wp": wp_np_b[b],
                "ct": ct_np,
                "st": st_np,
                "mk": mk_np,
                "pm": pm_np,
            }
        )

    res = run_bass_kernel_spmd(nc, in_maps, core_ids=list(range(8)))
    global _LAST
    _LAST = res

    y = np.empty((B, T, C), dtype=np.float32)
    present_k = np.empty((B, T, H, D), dtype=np.float32)
    present_v = np.empty((B, T, H, D), dtype=np.float32)
    inv = np.argsort(perm)
    for core in range(8):
        b, g = core // 4, core % 4
        r = res.results[core]
        y[b, g * TBLK : (g + 1) * TBLK, :] = r["y_t"].T
        pk = r["pk"].astype(np.float32)  # (256, T) rope-layout
        for u in range(4):
            h = 4 * g + u
            blk, off = u // 2, (u % 2) * 64
            kh = pk[blk * P + off : blk * P + off + 64, :]  # (64, T) [r|i]
            present_k[b, :, h, :] = kh.T[:, inv]
        pv = r["pv"]  # (T, 256) fp32
        present_v[b, :, 4 * g : 4 * g + 4, :] = pv.reshape(T, 4, D)
    return (y, present_k, present_v)
